# revision 1
# baseline (speedup 1.0000x reference)
"""Trainium2 Bass kernel for nn_ESBN_77352361001553 (scatter_memory).

Math being computed (see the reference's own faithfulness note): the conv
encoder output is dead code, and the LSTM input is constant zeros, so the
gate pre-activation contribution from the input is the constant bih + bhh
for every step and every batch element. Every batch row therefore follows
the identical 16-step, 512-dim LSTM trajectory from zero state, and the
(16, 1024, 4) output is out_t = Wo @ h_t + bo broadcast across batch.

Sharding: pure data parallelism over the batch dim — each of the 8 cores
owns a 128-wide batch shard. Each core runs the recurrence on-chip:
 - gates matvec on the PE as 64 (LDWEIGHTS, MATMUL N=1) pairs per step in
   fp16 (FWL fast-weight-load path, ~27 ns/pair), gate groups accumulating
   into four separate PSUM banks so each group's bias-add + activation
   overlaps the remaining groups' matmuls,
 - sigmoid/tanh on the ACT engine, state updates on the DVE,
 - output head as 4 accumulating matmuls + bias add, broadcast on-chip to
   a [16, 512] tile (stride-0 read) so the output DMA writes 16
   contiguous 2 KB packets.
Host code only re-lays-out the tiny weights and concatenates shards.
"""

import os

import numpy as np

T = 16
HID = 512
N_CORES = 8
BSH = 128  # batch shard per core

_BUILT = {}
last_results = None  # BassKernelResults of the most recent run (for tooling)


def _ensure_ntff_hook():
    """Register the axon NTFF profiling hook if the container lacks
    antenv.axon_hooks (slim boot). Mirrors trn_boot._ntff_profile_via_ctypes."""
    import contextlib
    import ctypes
    import sys
    import types

    try:
        from antenv.axon_hooks import get_axon_ntff_profile_hook  # noqa: F401

        return
    except ImportError:
        pass

    so_path = "/opt/axon/libaxon_pjrt.so"
    hook = None
    if os.path.exists(so_path):
        lib = ctypes.CDLL(so_path)
        if hasattr(lib, "axon_start_nrt_profile"):
            lib.axon_start_nrt_profile.argtypes = [
                ctypes.POINTER(ctypes.c_int64),
                ctypes.c_size_t,
            ]
            lib.axon_start_nrt_profile.restype = ctypes.c_int64
            lib.axon_stop_nrt_profile.argtypes = [ctypes.c_char_p]
            lib.axon_stop_nrt_profile.restype = ctypes.c_int64

            @contextlib.contextmanager
            def _hook(output_dir, device_ids):
                import jax

                jax.devices()  # force PJRT init so the .so's client exists
                if device_ids:
                    ids = (ctypes.c_int64 * len(device_ids))(*device_ids)
                    rc = lib.axon_start_nrt_profile(ids, len(device_ids))
                else:
                    rc = lib.axon_start_nrt_profile(None, 0)
                if rc != 0:
                    raise RuntimeError(f"axon_start_nrt_profile rc={rc}")
                try:
                    yield
                finally:
                    n = lib.axon_stop_nrt_profile(str(output_dir).encode())
                    print(f"ntff profile: {n} file(s) -> {output_dir}", file=sys.stderr)

            hook = _hook

    mod = types.ModuleType("antenv.axon_hooks")
    mod.get_axon_ntff_profile_hook = lambda: hook
    mod.set_axon_ntff_profile_hook = lambda h: None
    import antenv

    antenv.axon_hooks = mod
    sys.modules["antenv.axon_hooks"] = mod


def _build(nsteps=T):
    """Assemble the Bass module (one NeuronCore program, SPMD across 8)."""
    import concourse.bacc as bacc
    import concourse.bass as bass
    import concourse.mybir as mybir
    from concourse import tile

    f32 = mybir.dt.float32
    f16 = mybir.dt.float16
    AF = mybir.ActivationFunctionType

    nc = bacc.Bacc("TRN2", target_bir_lowering=False, debug=False, enable_asserts=False)

    # woT is packed into the tail columns of wT (both fp16); bo (replicated to
    # 128 rows) is packed into the tail columns of cst
    wT_d = nc.dram_tensor("wT", [128, 8208], f16, kind="ExternalInput")
    cst_d = nc.dram_tensor("cst", [128, 20], f32, kind="ExternalInput")
    out_d = nc.dram_tensor("out", [T, BSH, 4], f32, kind="ExternalOutput")


    with tile.TileContext(nc) as tc:
        with (
            tc.tile_pool(name="w", bufs=1) as wp,
            tc.tile_pool(name="st", bufs=1) as sp,
            tc.tile_pool(name="tmp", bufs=8) as tp,
            tc.tile_pool(name="ps", bufs=1, space="PSUM") as pp,
            tc.tile_pool(name="psd", bufs=1, space="PSUM") as pdp,
        ):
            wT = wp.tile([128, 8208], f16)
            cstb = sp.tile([128, 20], f32)
            cst = cstb[:, 0:16]
            woT = wT[:, 8192:8208]
            bo16 = cstb[0:16, 16:20]
            # preload both ACT function tables while the DMAs stream in
            warm = tp.tile([1, 1], f32, tag="warm")
            nc.vector.memset(warm[:], 0.0)
            warm2 = tp.tile([1, 1], f32, tag="warm2")
            nc.scalar.activation(warm2[:], warm[:], AF.Sigmoid)
            nc.scalar.activation(warm2[:], warm[:], AF.Tanh)
            # cst first (step 0 depends only on it), then the weights in two
            # halves on the same queue — a single dma_start already fans out
            # across the DMA engines internally, so the split only gives an
            # earlier completion signal for the first half
            nc.sync.dma_start(cstb[:], cst_d[:])
            nc.sync.dma_start(wT[:, 0:4096], wT_d[:, 0:4096])
            nc.sync.dma_start(wT[:, 4096:8208], wT_d[:, 4096:8208])

            # "Landing" ops: give each DMA-loaded tensor a first consumer per
            # engine with no other cross-engine deps, so downstream
            # instructions carry single sync-waits (no event-semaphore
            # legalization on the hot path).
            land = tp.tile([128, 1], f32, tag="land")
            nc.vector.tensor_copy(land[:], cstb[:, 0:1])
            one16 = sp.tile([128, 1], f16)
            nc.vector.memset(one16[:], 1.0)
            psd = pdp.tile([128, 1], f32, tag="dummy")

            def chunk_land(c):
                # pace the PE on a weight-half's arrival (single queue-sem wait)
                nc.tensor.matmul(
                    psd[:],
                    wT[:, c * 4096 : c * 4096 + 128],
                    one16[:],
                    start=True,
                    stop=True,
                    skip_group_check=True,
                )

            # h_t history, fp16, column 4t+ko holds h_t[ko*128 + p]
            hs = sp.tile([128, 4 * T], f16)
            cx = sp.tile([128, 4], f32)

            # per-gate-group PSUM banks (column order g | f | i | o) so the
            # per-group bias-add + activation can overlap the remaining
            # groups' matmuls (no PSUM bank conflict)
            psg = [pp.tile([128, 4], f32, tag=f"ps{n}", name=f"psg{n}") for n in range(4)]

            def step0():
                tg = tp.tile([128, 4], f32, tag="tg")
                sio = tp.tile([128, 8], f32, tag="sio")
                th = tp.tile([128, 4], f32, tag="th")
                nc.scalar.activation(tg[:], cst[:, 4:8], AF.Tanh)
                nc.scalar.activation(sio[:], cst[:, 8:16], AF.Sigmoid)
                nc.vector.tensor_mul(cx[:], sio[:, 0:4], tg[:])
                nc.scalar.activation(th[:], cx[:], AF.Tanh)
                nc.vector.tensor_mul(hs[:, 0:4], sio[:, 4:8], th[:])

            step0()  # step 0: gates == constant, no matvec needed

            def mm_group(t, gi):
                if t == 1 and gi in (0, 2):
                    # step 1 runs while the weight halves are still streaming
                    # in: pace the PE on each half's arrival
                    chunk_land(gi // 2)
                for c in range(4):
                    jo = 4 * gi + c
                    for ko in range(4):
                        tile_i = jo * 4 + ko
                        nc.tensor.matmul(
                            psg[gi][:, c : c + 1],
                            wT[:, tile_i * 128 : tile_i * 128 + 128],
                            hs[:, 4 * (t - 1) + ko : 4 * (t - 1) + ko + 1],
                            start=(ko == 0),
                            stop=(ko == 3),
                        )

            for t in range(1, nsteps):
                # matmuls in group order f, g, i, o (columns laid out in that
                # order); each group's bias-add + activation overlap the later
                # groups' matmuls, so only o's short chain trails the last MM
                gadd = [
                    tp.tile([128, 4], f32, tag=f"ga{n}", name=f"gadd{n}")
                    for n in range(4)
                ]
                tg = tp.tile([128, 4], f32, tag="tg")
                sf = tp.tile([128, 4], f32, tag="sf")
                si = tp.tile([128, 4], f32, tag="si")
                so = tp.tile([128, 4], f32, tag="so")
                th = tp.tile([128, 4], f32, tag="th")
                t1 = tp.tile([128, 4], f32, tag="t1")

                for gi in range(4):
                    mm_group(t, gi)
                # DVE adds + products, ordered so each runs as soon as its
                # group's stop-matmul drains; sigma_o reads PSUM directly
                nc.vector.tensor_add(gadd[0][:], psg[0][:], cst[:, 0:4])
                nc.scalar.activation(sf[:], gadd[0][:], AF.Sigmoid)
                nc.vector.tensor_add(gadd[1][:], psg[1][:], cst[:, 4:8])
                nc.scalar.activation(tg[:], gadd[1][:], AF.Tanh)
                nc.vector.tensor_mul(cx[:], sf[:], cx[:])
                nc.vector.tensor_add(gadd[2][:], psg[2][:], cst[:, 8:12])
                nc.scalar.activation(si[:], gadd[2][:], AF.Sigmoid)
                nc.vector.tensor_add(gadd[3][:], psg[3][:], cst[:, 12:16])
                nc.scalar.activation(so[:], gadd[3][:], AF.Sigmoid)
                nc.vector.tensor_mul(t1[:], si[:], tg[:])
                nc.vector.tensor_add(cx[:], cx[:], t1[:])
                nc.scalar.activation(th[:], cx[:], AF.Tanh)
                nc.vector.tensor_mul(hs[:, 4 * t : 4 * t + 4], so[:], th[:])

            # head: hps[t, d] = sum_k Wo[d, k] h_t[k]
            hps = pdp.tile([16, 4], f32, tag="head")
            for ko in range(4):
                nc.tensor.matmul(
                    hps[:],
                    hs[:, ko : ko + 4 * (T - 1) + 1 : 4],  # lhsT [K=128, M=16 steps]
                    woT[:, 4 * ko : 4 * ko + 4],  # rhs [K=128, N=4]
                    start=(ko == 0),
                    stop=(ko == 3),
                )
            head = sp.tile([16, 4], f32)
            nc.vector.tensor_add(head[:], hps[:], bo16[:])
            # broadcast on-chip to [16, 512]: partition t holds out_t repeated
            # 128x, so the output DMA writes 16 contiguous 2 KB packets
            bc = sp.tile([16, 512], f32)
            hap = head[:]
            rep = bass.AP(hap.tensor, hap.offset, [list(hap.ap[0]), [0, BSH], [1, 4]])
            nc.vector.tensor_copy(
                bc[:].rearrange("t (b d) -> t b d", d=4), rep
            )
            nc.sync.dma_start(
                out_d.rearrange("t b d -> t (b d)"),
                bc[:],
            )
    nc.compile()
    return nc


def prep_inputs(Whh, bih, bhh, Wo, bo):
    """Host-side weight relayout (all tensors are tiny: <5 MB total)."""
    Whh = np.asarray(Whh, np.float32)
    c = np.asarray(bih, np.float32) + np.asarray(bhh, np.float32)
    Wo = np.asarray(Wo, np.float32)
    bo = np.asarray(bo, np.float32)
    H = HID
    # reorder gate blocks from torch's i,f,g,o to f,g,i,o: earlier groups'
    # matmuls finish first, so their activations overlap later groups' matmuls
    perm = np.concatenate(
        [
            np.arange(H, 2 * H),
            np.arange(2 * H, 3 * H),
            np.arange(0, H),
            np.arange(3 * H, 4 * H),
        ]
    )
    Wp = Whh[perm]
    cp = c[perm]
    # tile-major interleave: tile (jo, ko) occupies columns (jo*4+ko)*128,
    # so step 1's matmuls consume the chunked DMA stream in arrival order
    wTm = np.ascontiguousarray(
        Wp.reshape(16, 128, 4, 128).transpose(3, 0, 2, 1).reshape(128, 8192)
    ).astype(np.float16)
    woT = np.ascontiguousarray(
        Wo.reshape(4, 4, 128).transpose(2, 1, 0).reshape(128, 16)
    ).astype(np.float16)
    wT = np.concatenate([wTm, woT], axis=1)  # (128, 8208)
    cstm = np.ascontiguousarray(cp.reshape(16, 128).T).astype(np.float32)
    bo128 = np.tile(bo, (128, 1)).astype(np.float32)
    cst = np.concatenate([cstm, bo128], axis=1)  # (128, 20)
    return {"wT": wT, "cst": cst}


def kernel(**inputs) -> np.ndarray:
    global last_results
    from concourse.bass_utils import run_bass_kernel_spmd

    if "nc" not in _BUILT:
        _BUILT["nc"] = _build()
    nc = _BUILT["nc"]

    in_map = prep_inputs(
        inputs["Whh"], inputs["bih"], inputs["bhh"], inputs["Wo"], inputs["bo"]
    )
    if os.environ.get("BASS_TRACE"):
        _ensure_ntff_hook()
    in_maps = [dict(in_map) for _ in range(N_CORES)]
    res = run_bass_kernel_spmd(
        nc,
        in_maps,
        core_ids=list(range(N_CORES)),
        trace=bool(os.environ.get("BASS_TRACE")),
    )
    last_results = res
    # gather: concatenate the 8 per-core batch shards
    return np.concatenate([r["out"] for r in res.results], axis=1)



# revision 32
# speedup vs baseline: 1.0560x; 1.0560x over previous
"""Trainium2 Bass kernel for nn_ESBN_77352361001553 (scatter_memory).

Math: the conv encoder is dead code and the LSTM input is constant zeros, so
every batch row follows the identical 16-step, 512-dim LSTM trajectory from
zero state. Output (16, 1024, 4) = broadcast of out_t = Wo @ h_t + bo across
the batch; each of the 8 cores produces the same (16, 512)-f32 row block and
the host reshapes/concats to (16, 1024, 4).

Hand-scheduled raw-bass kernel (no Tile framework):
 - tanh-only gates: host pre-scales f/i/o rows of Whh by 0.5 so
   tanh(pre) = 2*sigmoid(gate)-1, and tracks D = 2*cx, h2 = 2*h (the factor
   is folded into Whh's columns and Wo). Per step ACT runs 4 instrs
   (tanh_f, tanh_{g,i}, tanh_o, tanh(D/2)); DVE runs 4 fused
   scalar_tensor_tensor ops:
     u = (1+tf)*D ; v = (1+ti)*tg ; D' = 0.5*u + v ; h2 = (1+to)*th
 - gate biases enter PSUM via a K=4 (bias^T, I4) matmul opening each bank's
   accumulation group (no bias-add on the critical path).
 - four PSUM banks (f,g,i,o) so ACT reads never collide with PE writes.
 - head computed in two [8,4] PSUM chunks (steps 0-7 during step 9, steps
   8-15 at the end) so the output DMA mostly overlaps compute.
 - hand-placed semaphores; every instruction carries at most one wait, so no
   event-semaphore legalization and near-zero teardown.
"""

import os

import numpy as np

T = 16
HID = 512
N_CORES = 8
BSH = 128  # batch shard per core

WDT = os.environ.get("KERNEL_WDT", "f16")  # "f16" or "f8" (float8e3 = e3m4)
WSCALE = 16.0 if WDT == "f8" else 1.0
N_WARM_MM = int(os.environ.get("KERNEL_WARM", "56"))

_BUILT = {}
last_results = None  # BassKernelResults of the most recent run (for tooling)


def _ensure_ntff_hook():
    """Register the axon NTFF profiling hook if the container lacks
    antenv.axon_hooks (slim boot)."""
    import contextlib
    import ctypes
    import sys
    import types

    try:
        from antenv.axon_hooks import get_axon_ntff_profile_hook  # noqa: F401

        return
    except ImportError:
        pass

    so_path = "/opt/axon/libaxon_pjrt.so"
    hook = None
    if os.path.exists(so_path):
        lib = ctypes.CDLL(so_path)
        if hasattr(lib, "axon_start_nrt_profile"):
            lib.axon_start_nrt_profile.argtypes = [
                ctypes.POINTER(ctypes.c_int64),
                ctypes.c_size_t,
            ]
            lib.axon_start_nrt_profile.restype = ctypes.c_int64
            lib.axon_stop_nrt_profile.argtypes = [ctypes.c_char_p]
            lib.axon_stop_nrt_profile.restype = ctypes.c_int64

            @contextlib.contextmanager
            def _hook(output_dir, device_ids):
                import jax

                jax.devices()
                if device_ids:
                    ids = (ctypes.c_int64 * len(device_ids))(*device_ids)
                    rc = lib.axon_start_nrt_profile(ids, len(device_ids))
                else:
                    rc = lib.axon_start_nrt_profile(None, 0)
                if rc != 0:
                    raise RuntimeError(f"axon_start_nrt_profile rc={rc}")
                try:
                    yield
                finally:
                    n = lib.axon_stop_nrt_profile(str(output_dir).encode())
                    print(f"ntff profile: {n} file(s) -> {output_dir}", file=sys.stderr)

            hook = _hook

    mod = types.ModuleType("antenv.axon_hooks")
    mod.get_axon_ntff_profile_hook = lambda: hook
    mod.set_axon_ntff_profile_hook = lambda h: None
    import antenv

    antenv.axon_hooks = mod
    sys.modules["antenv.axon_hooks"] = mod


# ---------------------------------------------------------------------------
# Semaphore count schedule (precomputed; emission asserts it matches).
#
# DVE incs (every DVE instr incs sem_dve):
#   v0=1, hs0=2; per t in 1..15: u,v,Dp,hs = base+1..base+4 with
#   base = 2 + 4*(t-1) + (1 if t >= 10 else 0)   [bc1 sits after hs(9)]
# ACT incs: tq0=1, th0=2; per t: tqf=4t-1, tqgi=4t, tqo=4t+1, th=4t+2
# PE incs (only f/i/o stops + head chunks):
#   t<=9:  f=3t-2, i=3t-1, o=3t ; head1=28
#   t>=10: f=3t-1, i=3t,   o=3t+1 ; head2=47
#
# NOTE: the DVE does NOT interlock same-engine read-after-write — an
# instruction reading the previous instruction's output must carry an
# explicit wait on the DVE's own semaphore (D' below).
# ---------------------------------------------------------------------------
def _dve_base(t):
    return 2 + 4 * (t - 1) + (1 if t >= 10 else 0)


DVE_V0, DVE_HS0 = 1, 2
DVE_U = {t: _dve_base(t) + 1 for t in range(1, T)}
DVE_V = {t: _dve_base(t) + 2 for t in range(1, T)}
DVE_DP = {t: _dve_base(t) + 3 for t in range(1, T)}
DVE_HS = {t: _dve_base(t) + 4 for t in range(1, T)}
DVE_BC1 = DVE_HS[9] + 1  # 39
DVE_BC2 = DVE_HS[15] + 1  # 64

ACT_TQ0, ACT_TH0 = 1, 2
ACT_TQF = {t: 4 * t - 1 for t in range(1, T)}
ACT_TQGI = {t: 4 * t for t in range(1, T)}
ACT_TQO = {t: 4 * t + 1 for t in range(1, T)}
ACT_TH = {t: 4 * t + 2 for t in range(1, T)}

PE_F = {t: (3 * t - 2 if t <= 9 else 3 * t - 1) for t in range(1, T)}
PE_I = {t: (3 * t - 1 if t <= 9 else 3 * t) for t in range(1, T)}
PE_O = {t: (3 * t if t <= 9 else 3 * t + 1) for t in range(1, T)}
PE_HEAD1 = 28
PE_HEAD2 = 47


def _build():
    from contextlib import ExitStack

    import concourse.bacc as bacc
    import concourse.bass as bass
    import concourse.mybir as mybir

    f32 = mybir.dt.float32
    f16 = mybir.dt.float16
    wdt = mybir.dt.float8e3 if WDT == "f8" else f16
    AF = mybir.ActivationFunctionType
    ADD = mybir.AluOpType.add
    MUL = mybir.AluOpType.mult

    nc = bacc.Bacc("TRN2", target_bir_lowering=False, debug=False, enable_asserts=False)

    wT_d = nc.dram_tensor("wT", [128, 8192], wdt, kind="ExternalInput")
    woT_d = nc.dram_tensor("woT", [128, 16], f16, kind="ExternalInput")
    aux_d = nc.dram_tensor("aux", [8, 532], f16, kind="ExternalInput")
    cst_d = nc.dram_tensor("cst", [128, 16], f32, kind="ExternalInput")
    out_d = nc.dram_tensor("out", [16, 512], f32, kind="ExternalOutput")

    es = ExitStack()
    wT = es.enter_context(nc.sbuf_tensor("wTs", [128, 8192], wdt))
    woT = es.enter_context(nc.sbuf_tensor("woTs", [128, 16], f16))
    aux = es.enter_context(nc.sbuf_tensor("auxs", [8, 532], f16))
    cst = es.enter_context(nc.sbuf_tensor("csts", [128, 16], f32))
    tqt = es.enter_context(nc.sbuf_tensor("tqt", [128, 16], f32))
    tht = es.enter_context(nc.sbuf_tensor("tht", [128, 4], f32))
    ut = es.enter_context(nc.sbuf_tensor("ut", [128, 4], f32))
    vt = es.enter_context(nc.sbuf_tensor("vt", [128, 4], f32))
    Dt = es.enter_context(nc.sbuf_tensor("Dt", [128, 4], f32))
    hs = es.enter_context(nc.sbuf_tensor("hss", [128, 4 * T], f16))
    bco1 = es.enter_context(nc.sbuf_tensor("bco1", [8, 512], f32))
    bco2 = es.enter_context(nc.sbuf_tensor("bco2", [8, 512], f32))
    warm = es.enter_context(nc.sbuf_tensor("warms", [1, 2], f32))
    dumw = es.enter_context(nc.sbuf_tensor("dumw", [128, 128], f16))

    # GATES spans banks 0-2: f in bank 0 cols 0:4, g+i in bank 1 cols 0:8
    # (GATES cols 512:520), o in bank 2 cols 0:4 (GATES cols 1024:1028).
    # ACT reads a bank only after all its matmuls stopped; PE is then writing
    # a different bank, so no PSUM R/W collisions.
    GATES = nc.place_psum_tensor("GATES", [128, 1536], f32, bank=0)
    H1 = nc.place_psum_tensor("H1", [8, 4], f32, bank=3)
    H2 = nc.place_psum_tensor("H2", [8, 4], f32, bank=4)
    DUM = nc.place_psum_tensor("DUM", [128, 1], f32, bank=5)

    # one semaphore per input DMA: completions of different DMAs interleave
    # across the 16 SDMA engines, so a shared counter cannot order them
    sem_dw = [es.enter_context(nc.semaphore(f"sem_dw{k}")) for k in range(5)]
    sem_aux = es.enter_context(nc.semaphore("sem_aux"))
    sem_cst = es.enter_context(nc.semaphore("sem_cst"))
    sem_do = es.enter_context(nc.semaphore("sem_do"))
    sem_pe = es.enter_context(nc.semaphore("sem_pe"))
    sem_act = es.enter_context(nc.semaphore("sem_act"))
    sem_dve = es.enter_context(nc.semaphore("sem_dve"))

    # PSUM column of gate group g (0=f, 1=g, 2=i, 3=o), tile column c
    GCOL0 = {0: 0, 1: 512, 2: 516, 3: 1024}

    def gates_cols(g, c0, n):
        return GATES[:, GCOL0[g] + c0 : GCOL0[g] + c0 + n]

    inv_s = 1.0 / WSCALE

    with nc.Block() as block:

        @block.tensor
        def _(tensor):
            pe_cnt = 0
            # --- HAM warm-up: dummy pairs on the dummy bank while DMA runs
            for _i in range(N_WARM_MM):
                tensor.matmul(
                    DUM[:, 0:1],
                    dumw[:, 0:128],
                    dumw[:, 0:1],
                    start=True,
                    stop=True,
                    skip_group_check=True,
                )

            def bias_f():
                tensor.matmul(
                    GATES[:, 0:4], aux[0:4, 20:148], aux[0:4, 0:4],
                    start=True, stop=False, skip_group_check=True,
                )

            def bias_gi():
                tensor.matmul(
                    GATES[:, 512:520], aux[0:8, 148:276], aux[0:8, 0:8],
                    start=True, stop=False, skip_group_check=True,
                )

            def bias_o():
                tensor.matmul(
                    GATES[:, 1024:1028], aux[0:4, 276:404], aux[0:4, 0:4],
                    start=True, stop=False, skip_group_check=True,
                )

            def group_mms(t, g, inc=True, stop=True):
                nonlocal pe_cnt
                for c in range(4):
                    for ko in range(4):
                        tile = g * 16 + c * 4 + ko
                        mm = tensor.matmul(
                            gates_cols(g, c, 1),
                            wT[:, tile * 128 : tile * 128 + 128],
                            hs[:, 4 * (t - 1) + ko : 4 * (t - 1) + ko + 1],
                            start=False,
                            stop=(stop and c == 3 and ko == 3),
                            skip_group_check=True,
                        )
                if inc:
                    mm.then_inc(sem_pe)
                    pe_cnt += 1
                return pe_cnt

            def head_mms(trange, HB, mark):
                nonlocal pe_cnt
                t0 = trange[0]
                for ko in range(4):
                    tensor.matmul(
                        HB[0:8, 0:4],
                        hs[:, 4 * t0 + ko : 4 * t0 + ko + 29 : 4],  # [128, 8]
                        woT[:, 4 * ko : 4 * ko + 4],
                        start=(ko == 0),
                        stop=False,
                        skip_group_check=True,
                    )
                mmb = tensor.matmul(
                    HB[0:8, 0:4],
                    aux[0:1, 8:16],
                    aux[0:1, 16:20],
                    start=False,
                    stop=True,
                    skip_group_check=True,
                )
                mmb.then_inc(sem_pe)
                pe_cnt += 1
                assert pe_cnt == mark, (pe_cnt, mark)

            for t in range(1, T):
                if t == 1:
                    tensor.wait_ge(sem_aux, 16)  # identities + biasT
                else:
                    tensor.wait_ge(sem_dve, DVE_HS[t - 1])
                bias_f()
                if t == 1:
                    tensor.wait_ge(sem_dw[0], 16)
                    tensor.wait_ge(sem_dve, DVE_HS0)
                c = group_mms(t, 0)
                assert c == PE_F[t], (t, c, PE_F[t])
                bias_gi()
                if t == 1:
                    tensor.wait_ge(sem_dw[1], 16)
                group_mms(t, 1, inc=False, stop=False)
                if t == 1:
                    tensor.wait_ge(sem_dw[2], 16)
                c = group_mms(t, 2)
                assert c == PE_I[t], (t, c, PE_I[t])
                bias_o()
                if t == 1:
                    tensor.wait_ge(sem_dw[3], 16)
                c = group_mms(t, 3)
                assert c == PE_O[t], (t, c, PE_O[t])
                if t == 9:
                    tensor.wait_ge(sem_dw[4], 16)  # woT
                    head_mms(range(0, 8), H1, PE_HEAD1)
            tensor.wait_ge(sem_dve, DVE_HS[15])
            head_mms(range(8, 16), H2, PE_HEAD2)

        @block.scalar
        def _(scalar):
            act_cnt = 0

            def act(out, in_, scale, wait=None):
                nonlocal act_cnt
                if wait is not None:
                    scalar.wait_ge(*wait)
                a = scalar.activation(out, in_, AF.Tanh, scale=scale)
                a.then_inc(sem_act)
                act_cnt += 1
                return act_cnt

            # warm the tanh table during the DMA window (input uninitialized;
            # only the table load matters, the output is never read)
            scalar.activation(warm[0:1, 1:2], warm[0:1, 0:1], AF.Tanh)
            # step 0: gates are the constant cst (true units)
            c = act(tqt[:, 0:16], cst[:, 0:16], 1.0, wait=(sem_cst, 16))
            assert c == ACT_TQ0
            c = act(tht[:, 0:4], Dt[:, 0:4], 0.5, wait=(sem_dve, DVE_V0))
            assert c == ACT_TH0
            for t in range(1, T):
                c = act(tqt[:, 0:4], GATES[:, 0:4], inv_s, wait=(sem_pe, PE_F[t]))
                assert c == ACT_TQF[t]
                act(tqt[:, 4:12], GATES[:, 512:520], inv_s, wait=(sem_pe, PE_I[t]))
                act(tqt[:, 12:16], GATES[:, 1024:1028], inv_s, wait=(sem_pe, PE_O[t]))
                c = act(tht[:, 0:4], Dt[:, 0:4], 0.5, wait=(sem_dve, DVE_DP[t]))
                assert c == ACT_TH[t]

        @block.vector
        def _(vector):
            dve_cnt = 0

            def stt(out, in0, scalar_imm, in1, op0, op1, wait=None):
                nonlocal dve_cnt
                if wait is not None:
                    vector.wait_ge(*wait)
                i = vector.scalar_tensor_tensor(out, in0, scalar_imm, in1, op0, op1)
                i.then_inc(sem_dve)
                dve_cnt += 1
                return dve_cnt

            tq_f = tqt[:, 0:4]
            tq_g = tqt[:, 4:8]
            tq_i = tqt[:, 8:12]
            tq_o = tqt[:, 12:16]

            # step 0: D0 = (1+ti)*tg ; h2_0 = (1+to)*th0
            c = stt(Dt[:, 0:4], tq_i, 1.0, tq_g, ADD, MUL, wait=(sem_act, ACT_TQ0))
            assert c == DVE_V0
            c = stt(hs[:, 0:4], tq_o, 1.0, tht[:, 0:4], ADD, MUL, wait=(sem_act, ACT_TH0))
            assert c == DVE_HS0

            def bc(HB, dst, mark_pe, mark_dve):
                nonlocal dve_cnt
                vector.wait_ge(sem_pe, mark_pe)
                base = HB[0:8, 0:4]
                rep = bass.AP(base.tensor, base.offset, [list(base.ap[0]), [0, 128], [1, 4]])
                i = vector.tensor_copy(dst.rearrange("p (b d) -> p b d", d=4), rep)
                i.then_inc(sem_dve)
                dve_cnt += 1
                assert dve_cnt == mark_dve, (dve_cnt, mark_dve)

            for t in range(1, T):
                c = stt(ut[:, 0:4], tq_f, 1.0, Dt[:, 0:4], ADD, MUL,
                        wait=(sem_act, ACT_TQF[t]))
                assert c == DVE_U[t]
                c = stt(vt[:, 0:4], tq_i, 1.0, tq_g, ADD, MUL,
                        wait=(sem_act, ACT_TQGI[t]))
                assert c == DVE_V[t]
                # D' = 0.5*u + v; reads the two preceding DVE outputs, so it
                # must wait for v's completion on the DVE's own semaphore
                c = stt(Dt[:, 0:4], ut[:, 0:4], 0.5, vt[:, 0:4], MUL, ADD,
                        wait=(sem_dve, DVE_V[t]))
                assert c == DVE_DP[t]
                c = stt(hs[:, 4 * t : 4 * t + 4], tq_o, 1.0, tht[:, 0:4], ADD, MUL,
                        wait=(sem_act, ACT_TH[t]))
                assert c == DVE_HS[t]
                if t == 9:
                    bc(H1, bco1[:], PE_HEAD1, DVE_BC1)
            bc(H2, bco2[:], PE_HEAD2, DVE_BC2)

        @block.sync
        def _(sync):
            sync.dma_start(aux[:], aux_d[:]).then_inc(sem_aux, 16)
            sync.dma_start(cst[:], cst_d[:]).then_inc(sem_cst, 16)
            for k in range(4):
                sync.dma_start(
                    wT[:, k * 2048 : (k + 1) * 2048],
                    wT_d[:, k * 2048 : (k + 1) * 2048],
                ).then_inc(sem_dw[k], 16)
            sync.dma_start(woT[:], woT_d[:]).then_inc(sem_dw[4], 16)
            sync.wait_ge(sem_dve, DVE_BC1)
            sync.dma_start(out_d[0:8, :], bco1[:]).then_inc(sem_do, 16)
            sync.wait_ge(sem_dve, DVE_BC2)
            sync.dma_start(out_d[8:16, :], bco2[:]).then_inc(sem_do, 16)
            sync.wait_ge(sem_do, 32)

    es.close()
    nc.compile()
    return nc


def prep_inputs(Whh, bih, bhh, Wo, bo):
    """Host-side weight relayout + tanh-reparameterization (all tiny)."""
    Whh = np.asarray(Whh, np.float64)
    b = np.asarray(bih, np.float64) + np.asarray(bhh, np.float64)
    Wo = np.asarray(Wo, np.float64)
    bo = np.asarray(bo, np.float64)
    H = HID

    # torch gate order i,f,g,o -> our group order f,g,i,o
    perm = np.concatenate(
        [np.arange(H, 2 * H), np.arange(2 * H, 3 * H),
         np.arange(0, H), np.arange(3 * H, 4 * H)]
    )
    Wp = Whh[perm]
    bp = b[perm]
    # rows: f,i,o scaled by 0.5 (tanh(x/2) = 2*sigmoid(x)-1); g unscaled
    rs = np.ones(4 * H)
    rs[0:H] = 0.5      # f
    rs[2 * H : 3 * H] = 0.5  # i
    rs[3 * H :] = 0.5  # o
    # columns: h2 = 2h -> fold 0.5 into columns
    W2 = rs[:, None] * Wp * 0.5
    b2 = rs * bp

    # gate weight tiles: tile (g,c,ko) at cols (g*16+c*4+ko)*128,
    # lhsT[k, m] = W2[512g+128c+m, 128ko+k]
    Wr = W2.reshape(4, 4, 128, 4, 128)  # [g, c, m, ko, k]
    wTm = np.ascontiguousarray(Wr.transpose(4, 0, 1, 3, 2).reshape(128, 8192))
    wTm = wTm * WSCALE
    if WDT == "f8":
        import ml_dtypes

        wT = wTm.astype(ml_dtypes.float8_e3m4)
    else:
        wT = wTm.astype(np.float16)

    # head: out_t = Wo' @ h2 + bo with Wo' = 0.5*Wo
    Wo2 = 0.5 * Wo  # (4, 512)
    woT = np.ascontiguousarray(
        Wo2.reshape(4, 4, 128).transpose(2, 1, 0).reshape(128, 16)
    ).astype(np.float16)

    # aux [8, 532] f16:
    #   [0:8, 0:8]     I8 (top-left 4x4 doubles as I4)
    #   [0,   8:16]    ones8 (head bias lhsT)
    #   [0,   16:20]   bo
    #   [0:4, 20:148]  biasT for f bank    (scaled by WSCALE)
    #   [0:8, 148:276] biasT for g+i bank
    #   [0:4, 276:404] biasT for o bank
    auxm = np.zeros((8, 532), np.float64)
    auxm[0:8, 0:8] = np.eye(8)
    auxm[0, 8:16] = 1.0
    auxm[0, 16:20] = bo
    bs = (WSCALE * b2).reshape(4, 4, 128)  # [group(f,g,i,o), col, dim]
    auxm[0:4, 20:148] = bs[0]
    auxm[0:4, 148:276] = bs[1]
    auxm[4:8, 148:276] = bs[2]
    auxm[0:4, 276:404] = bs[3]
    aux = auxm.astype(np.float16)

    # cst: step-0 ACT input in true units: [128, 16], col j = dims 128j..128j+127
    cstm = np.ascontiguousarray(b2.reshape(16, 128).T).astype(np.float32)
    return {"wT": wT, "woT": woT, "aux": aux, "cst": cstm}


def kernel(**inputs) -> np.ndarray:
    global last_results
    from concourse.bass_utils import run_bass_kernel_spmd

    if "nc" not in _BUILT:
        _BUILT["nc"] = _build()
    nc = _BUILT["nc"]

    in_map = prep_inputs(
        inputs["Whh"], inputs["bih"], inputs["bhh"], inputs["Wo"], inputs["bo"]
    )
    if os.environ.get("BASS_TRACE"):
        _ensure_ntff_hook()
    in_maps = [dict(in_map) for _ in range(N_CORES)]
    res = run_bass_kernel_spmd(
        nc,
        in_maps,
        core_ids=list(range(N_CORES)),
        trace=bool(os.environ.get("BASS_TRACE")),
    )
    last_results = res
    # out rows: [16, 512] -> (16, 128, 4); concat batch shards
    return np.concatenate(
        [r["out"].reshape(T, BSH, 4) for r in res.results], axis=1
    )
